# revision 12
# baseline (speedup 1.0000x reference)
"""Trainium2 Bass kernel for nn_BackProjLayer.

Math: reference computes, per sample n,
    eigh(S) -> (lam, V);  G = V @ diag(sqrt(max(lam,0)));  y = D^H G
    out[n,p] = sum_d |y[p,d]|^2 - tau[p] = [D^H S_plus D]_pp - tau[p]
Since S = A A^H / Nch is Hermitian PSD by construction, S_plus == S up to
float32 eigensolver noise, so
    out[n,p] = Re(d_p^H S[n] d_p) - tau[p]
With S = Sr + i Si (Sr sym, Si antisym) and d = dr + i di:
    Re(d^H S d) = sum_{c,c'} Sr[c,c'] (dr[c]dr[c'] + di[c]di[c'])
                           + Si[c,c'] (di[c]dr[c'] - dr[c]di[c'])
i.e. a (N,33) @ (33,242) matmul with features
    X[n] = [Sr[n].ravel(), Si[n].ravel(), 1.0]
and weights
    W = [Wr(16,242); Wi(16,242); -tau]   built from D_re, D_im, tau.

Sharding: pure data parallel over N across 8 cores (8192 samples/core).
Per core the device kernel is: for each 128-sample tile,
PSUM[128,242] = XT_tile(33,128).T @ W(33,242), copy PSUM->SBUF, DMA out.
"""

import sys

for _p in ("/opt/trn_rl_repo", "/root/.axon_site/_ro/trn_rl_repo"):
    if _p not in sys.path:
        sys.path.insert(0, _p)

import numpy as np

N_SAMPLES = 65536
N_CH = 4
N_PX = 242
N_CORES = 8
N_LOC = N_SAMPLES // N_CORES  # 8192

K_FEAT = 2 * N_CH * N_CH + 1  # 33

TILE = 128
N_TILES = N_LOC // TILE  # 64
N_OUT_DMAS = 8  # one per HWDGE lane

# matmul config: use_f32r streams fp32 through the PE at 1 col/cycle
# (needs moving free dim >= 256, hence the padded weight width)
USE_F32R = False
PXP = 256 if USE_F32R else N_PX

_BUILT = {}


def _build_nc():
    import concourse.bass as bass
    import concourse.mybir as mybir
    from concourse import bacc
    from concourse.bass import ts
    from concourse.tile import TileContext

    f32 = mybir.dt.float32
    f32r = mybir.dt.float32r

    # Bacc (not plain Bass): its compile() lowers multi-wait sync_infos into
    # chained EventSemaphores (TRN2 allows 1 wait/instruction) — walrus
    # rejects the raw Tile output otherwise.
    nc = bacc.Bacc("TRN2", target_bir_lowering=False, debug=False)
    # xTw packs the weight matrix (first PXP columns) and the transposed
    # feature matrix (next N_LOC columns) so the whole input arrives in a
    # single DMA stream: each matmul then waits on at most one DMA
    # semaphore (walrus rejects matmuls with too many sync waits).
    xTw = nc.declare_dram_parameter("xTw", [K_FEAT, PXP + N_LOC], f32, isOutput=False)
    out = nc.declare_dram_parameter("out", [N_LOC, N_PX], f32, isOutput=True)

    def mm_ap(ap):
        return ap.bitcast(f32r) if USE_F32R else ap

    # Wait-budget constraint: this walrus build allows only ONE sync wait
    # per (non-sequencer) instruction. Structure:
    #   - input DMA on gpsimd (SWDGE) -> does not occupy a HWDGE lane
    #   - matmul waits on {input DMA (first) | psum slot release by ACT}
    #   - all PSUM->SBUF copies on the scalar engine (single monotonic tick)
    #   - exactly 8 grouped output DMAs on the 8 HWDGE lanes (no lane
    #     reuse), each waiting on one ACT tick (its group's last copy)
    TILES_PER_GROUP = N_TILES // N_OUT_DMAS  # 8
    out_g = out.rearrange("(g j p) c -> g p j c", p=TILE, j=TILES_PER_GROUP)

    with TileContext(nc) as tc:
        with (
            tc.tile_pool(name="xin", bufs=1) as xpool,
            tc.tile_pool(name="ps", bufs=4, space="PSUM") as pspool,
            tc.tile_pool(name="ob", bufs=1) as opool,
        ):
            xt0 = xpool.tile([K_FEAT, PXP + N_LOC], f32)
            nc.gpsimd.dma_start(xt0[:], xTw[:])
            wt = xt0[:, :PXP]

            for g in range(N_OUT_DMAS):
                gt = opool.tile([TILE, TILES_PER_GROUP * N_PX], f32, tag=f"g{g}")
                for j in range(TILES_PER_GROUP):
                    t = g * TILES_PER_GROUP + j
                    off = PXP + t * TILE
                    ps = pspool.tile([TILE, PXP], f32)
                    nc.tensor.matmul(
                        ps[:],
                        mm_ap(xt0[:, off : off + TILE]),
                        mm_ap(wt),
                        start=True,
                        stop=True,
                    )
                    nc.scalar.copy(gt[:, j * N_PX : (j + 1) * N_PX], ps[:, :N_PX])
                nc.sync.dma_start(
                    out_g[g],
                    gt[:].rearrange("p (j c) -> p j c", j=TILES_PER_GROUP),
                )

    nc.compile()
    return nc


def _get_nc():
    if "nc" not in _BUILT:
        _BUILT["nc"] = _build_nc()
    return _BUILT["nc"]


def _pack_host(S_re, S_im, D_re, D_im, tau):
    """Build per-core input maps: transposed feature matrix + weight matrix."""
    Dr = np.asarray(D_re, dtype=np.float32)
    Di = np.asarray(D_im, dtype=np.float32)
    tau = np.asarray(tau, dtype=np.float32)

    Wr = (Dr[:, None, :] * Dr[None, :, :] + Di[:, None, :] * Di[None, :, :])
    Wi = (Di[:, None, :] * Dr[None, :, :] - Dr[:, None, :] * Di[None, :, :])
    W = np.empty((K_FEAT, PXP), dtype=np.float32)
    W[:, N_PX:] = 0.0
    W[:16, :N_PX] = Wr.reshape(16, N_PX)
    W[16:32, :N_PX] = Wi.reshape(16, N_PX)
    W[32, :N_PX] = -tau

    X = np.empty((K_FEAT, N_SAMPLES), dtype=np.float32)
    X[:16] = np.asarray(S_re, dtype=np.float32).reshape(N_SAMPLES, 16).T
    X[16:32] = np.asarray(S_im, dtype=np.float32).reshape(N_SAMPLES, 16).T
    X[32] = 1.0

    in_maps = []
    for i in range(N_CORES):
        xtw = np.empty((K_FEAT, PXP + N_LOC), dtype=np.float32)
        xtw[:, :PXP] = W
        xtw[:, PXP:] = X[:, i * N_LOC : (i + 1) * N_LOC]
        in_maps.append({"xTw": xtw})
    return in_maps


def _run(inputs, trace=False):
    from concourse.bass_utils import run_bass_kernel_spmd

    nc = _get_nc()
    in_maps = _pack_host(**inputs)
    res = run_bass_kernel_spmd(
        nc, in_maps, list(range(N_CORES)), trace=trace
    )
    out = np.concatenate([res.results[i]["out"] for i in range(N_CORES)], axis=0)
    return out, res


def kernel(**inputs) -> np.ndarray:
    out, _ = _run(inputs, trace=False)
    return out


# revision 15
# speedup vs baseline: 1.2097x; 1.2097x over previous
"""Trainium2 Bass kernel for nn_BackProjLayer.

Math: reference computes, per sample n,
    eigh(S) -> (lam, V);  G = V @ diag(sqrt(max(lam,0)));  y = D^H G
    out[n,p] = sum_d |y[p,d]|^2 - tau[p] = [D^H S_plus D]_pp - tau[p]
Since S = A A^H / Nch is Hermitian PSD by construction, S_plus == S up to
float32 eigensolver noise, so
    out[n,p] = Re(d_p^H S[n] d_p) - tau[p]
With S = Sr + i Si (Sr sym, Si antisym) and d = dr + i di:
    Re(d^H S d) = sum_{c,c'} Sr[c,c'] (dr[c]dr[c'] + di[c]di[c'])
                           + Si[c,c'] (di[c]dr[c'] - dr[c]di[c'])
i.e. a (N,33) @ (33,242) matmul with features
    X[n] = [Sr[n].ravel(), Si[n].ravel(), 1.0]
and weights
    W = [Wr(16,242); Wi(16,242); -tau]   built from D_re, D_im, tau.

Sharding: pure data parallel over N across 8 cores (8192 samples/core).
Per core the device kernel is: for each 128-sample tile,
PSUM[128,242] = XT_tile(33,128).T @ W(33,242), copy PSUM->SBUF, DMA out.
"""

import sys

for _p in ("/opt/trn_rl_repo", "/root/.axon_site/_ro/trn_rl_repo"):
    if _p not in sys.path:
        sys.path.insert(0, _p)

import numpy as np

N_SAMPLES = 65536
N_CH = 4
N_PX = 242
N_CORES = 8
N_LOC = N_SAMPLES // N_CORES  # 8192

K_FEAT = 2 * N_CH * N_CH + 1  # 33

TILE = 128
N_TILES = N_LOC // TILE  # 64
N_OUT_DMAS = 8  # one per HWDGE lane

# matmul config: use_f32r streams fp32 through the PE at 1 col/cycle
# (needs moving free dim >= 256, hence the padded weight width)
USE_F32R = True
PXP = 256 if USE_F32R else N_PX

_BUILT = {}


def _build_nc():
    import concourse.bass as bass
    import concourse.mybir as mybir
    from concourse import bacc
    from concourse.bass import ts
    from concourse.tile import TileContext

    f32 = mybir.dt.float32
    f32r = mybir.dt.float32r

    # Bacc (not plain Bass): its compile() lowers multi-wait sync_infos into
    # chained EventSemaphores (TRN2 allows 1 wait/instruction) — walrus
    # rejects the raw Tile output otherwise.
    nc = bacc.Bacc("TRN2", target_bir_lowering=False, debug=False)
    # xTw packs the weight matrix (first PXP columns) and the transposed
    # feature matrix (next N_LOC columns). Declared float32r end-to-end
    # when USE_F32R (same 4-byte fp32 bits on the host side; the PE streams
    # f32r at 1 col/cycle vs 4 for plain fp32 when the moving dim >= 256).
    in_dt = f32r if USE_F32R else f32
    xTw = nc.declare_dram_parameter("xTw", [K_FEAT, PXP + N_LOC], in_dt, isOutput=False)
    out = nc.declare_dram_parameter("out", [N_LOC, N_PX], f32, isOutput=True)

    # Wait-budget constraint: this walrus build allows only ONE sync wait
    # per (non-sequencer) instruction. Structure:
    #   - input DMA on gpsimd (SWDGE) -> does not occupy a HWDGE lane
    #   - matmul waits on {input DMA (first) | psum slot release by ACT}
    #   - all PSUM->SBUF copies on the scalar engine (single monotonic tick)
    #   - exactly 8 grouped output DMAs on the 8 HWDGE lanes (no lane
    #     reuse), each waiting on one ACT tick (its group's last copy)
    TILES_PER_GROUP = N_TILES // N_OUT_DMAS  # 8
    out_g = out.rearrange("(g j p) c -> g p j c", p=TILE, j=TILES_PER_GROUP)

    with TileContext(nc) as tc:
        with (
            tc.tile_pool(name="xin", bufs=1) as xpool,
            tc.tile_pool(name="ps", bufs=4, space="PSUM") as pspool,
            tc.tile_pool(name="ob", bufs=1) as opool,
        ):
            xt0 = xpool.tile([K_FEAT, PXP + N_LOC], in_dt)
            nc.gpsimd.dma_start(xt0[:], xTw[:])
            wt = xt0[:, :PXP]

            for g in range(N_OUT_DMAS):
                gt = opool.tile([TILE, TILES_PER_GROUP * N_PX], f32, tag=f"g{g}")
                for j in range(TILES_PER_GROUP):
                    t = g * TILES_PER_GROUP + j
                    off = PXP + t * TILE
                    ps = pspool.tile([TILE, PXP], f32)
                    nc.tensor.matmul(
                        ps[:],
                        xt0[:, off : off + TILE],
                        wt,
                        start=True,
                        stop=True,
                    )
                    nc.scalar.copy(gt[:, j * N_PX : (j + 1) * N_PX], ps[:, :N_PX])
                nc.sync.dma_start(
                    out_g[g],
                    gt[:].rearrange("p (j c) -> p j c", j=TILES_PER_GROUP),
                )

    nc.compile()
    return nc


def _get_nc():
    if "nc" not in _BUILT:
        _BUILT["nc"] = _build_nc()
    return _BUILT["nc"]


def _pack_host(S_re, S_im, D_re, D_im, tau):
    """Build per-core input maps: transposed feature matrix + weight matrix."""
    Dr = np.asarray(D_re, dtype=np.float32)
    Di = np.asarray(D_im, dtype=np.float32)
    tau = np.asarray(tau, dtype=np.float32)

    Wr = (Dr[:, None, :] * Dr[None, :, :] + Di[:, None, :] * Di[None, :, :])
    Wi = (Di[:, None, :] * Dr[None, :, :] - Dr[:, None, :] * Di[None, :, :])
    W = np.empty((K_FEAT, PXP), dtype=np.float32)
    W[:, N_PX:] = 0.0
    W[:16, :N_PX] = Wr.reshape(16, N_PX)
    W[16:32, :N_PX] = Wi.reshape(16, N_PX)
    W[32, :N_PX] = -tau

    X = np.empty((K_FEAT, N_SAMPLES), dtype=np.float32)
    X[:16] = np.asarray(S_re, dtype=np.float32).reshape(N_SAMPLES, 16).T
    X[16:32] = np.asarray(S_im, dtype=np.float32).reshape(N_SAMPLES, 16).T
    X[32] = 1.0

    in_maps = []
    for i in range(N_CORES):
        xtw = np.empty((K_FEAT, PXP + N_LOC), dtype=np.float32)
        xtw[:, :PXP] = W
        xtw[:, PXP:] = X[:, i * N_LOC : (i + 1) * N_LOC]
        in_maps.append({"xTw": xtw})
    return in_maps


def _run(inputs, trace=False):
    from concourse.bass_utils import run_bass_kernel_spmd

    nc = _get_nc()
    in_maps = _pack_host(**inputs)
    res = run_bass_kernel_spmd(
        nc, in_maps, list(range(N_CORES)), trace=trace
    )
    out = np.concatenate([res.results[i]["out"] for i in range(N_CORES)], axis=0)
    return out, res


def kernel(**inputs) -> np.ndarray:
    out, _ = _run(inputs, trace=False)
    return out


# revision 16
# speedup vs baseline: 1.4858x; 1.2282x over previous
"""Trainium2 Bass kernel for nn_BackProjLayer.

Math: reference computes, per sample n,
    eigh(S) -> (lam, V);  G = V @ diag(sqrt(max(lam,0)));  y = D^H G
    out[n,p] = sum_d |y[p,d]|^2 - tau[p] = [D^H S_plus D]_pp - tau[p]
Since S = A A^H / Nch is Hermitian PSD by construction, S_plus == S up to
float32 eigensolver noise, so
    out[n,p] = Re(d_p^H S[n] d_p) - tau[p]
With S = Sr + i Si (Sr sym, Si antisym) and d = dr + i di:
    Re(d^H S d) = sum_{c,c'} Sr[c,c'] (dr[c]dr[c'] + di[c]di[c'])
                           + Si[c,c'] (di[c]dr[c'] - dr[c]di[c'])
i.e. a (N,33) @ (33,242) matmul with features
    X[n] = [Sr[n].ravel(), Si[n].ravel(), 1.0]
and weights
    W = [Wr(16,242); Wi(16,242); -tau]   built from D_re, D_im, tau.

Sharding: pure data parallel over N across 8 cores (8192 samples/core).
Per core the device kernel is: for each 128-sample tile,
PSUM[128,242] = XT_tile(33,128).T @ W(33,242), copy PSUM->SBUF, DMA out.
"""

import sys

for _p in ("/opt/trn_rl_repo", "/root/.axon_site/_ro/trn_rl_repo"):
    if _p not in sys.path:
        sys.path.insert(0, _p)

import numpy as np

N_SAMPLES = 65536
N_CH = 4
N_PX = 242
N_CORES = 8
N_LOC = N_SAMPLES // N_CORES  # 8192

K_FEAT = 2 * N_CH * N_CH + 1  # 33

TILE = 128
N_TILES = N_LOC // TILE  # 64
N_OUT_DMAS = 8  # one per HWDGE lane

# matmul config: use_f32r streams fp32 through the PE at 1 col/cycle
# (needs moving free dim >= 256, hence the padded weight width)
USE_F32R = True
PXP = 256 if USE_F32R else N_PX

_BUILT = {}


def _build_nc():
    import concourse.bass as bass
    import concourse.mybir as mybir
    from concourse import bacc
    from concourse.bass import ts
    from concourse.tile import TileContext

    f32 = mybir.dt.float32
    f32r = mybir.dt.float32r

    # Bacc (not plain Bass): its compile() lowers multi-wait sync_infos into
    # chained EventSemaphores (TRN2 allows 1 wait/instruction) — walrus
    # rejects the raw Tile output otherwise.
    nc = bacc.Bacc("TRN2", target_bir_lowering=False, debug=False)
    # xTw packs the weight matrix (first PXP columns) and the transposed
    # feature matrix (next N_LOC columns). Declared float32r end-to-end
    # when USE_F32R (same 4-byte fp32 bits on the host side; the PE streams
    # f32r at 1 col/cycle vs 4 for plain fp32 when the moving dim >= 256).
    in_dt = f32r if USE_F32R else f32
    xTw = nc.declare_dram_parameter("xTw", [K_FEAT, PXP + N_LOC], in_dt, isOutput=False)
    out = nc.declare_dram_parameter("out", [N_LOC, N_PX], f32, isOutput=True)

    # Pipeline (Bacc.compile() later splits any multi-wait instruction into
    # chained EventSemaphores, so cross-engine deps are unconstrained here):
    #   - chunked input DMAs on sync (HWDGE) so matmuls start early
    #   - 2 matmuls share one PSUM bank (at PXP-col offsets); one copy per
    #     bank moves both 242-col blocks, alternating scalar/vector engines
    #   - 8 grouped output DMAs (~1 MB each) on HWDGE
    TILES_PER_GROUP = N_TILES // N_OUT_DMAS  # 8
    out_g = out.rearrange("(g j p) c -> g p j c", p=TILE, j=TILES_PER_GROUP)
    X_CHUNK = 2048
    n_xchunks = N_LOC // X_CHUNK  # 4

    with TileContext(nc) as tc:
        with (
            tc.tile_pool(name="xin", bufs=1) as xpool,
            tc.tile_pool(name="ps", bufs=7, space="PSUM") as pspool,
            tc.tile_pool(name="ob", bufs=1) as opool,
        ):
            xt0 = xpool.tile([K_FEAT, PXP + N_LOC], in_dt)
            nc.sync.dma_start(xt0[:, : PXP + X_CHUNK], xTw[:, : PXP + X_CHUNK])
            for ci in range(1, n_xchunks):
                lo = PXP + ci * X_CHUNK
                nc.sync.dma_start(xt0[:, lo : lo + X_CHUNK], xTw[:, lo : lo + X_CHUNK])
            wt = xt0[:, :PXP]

            for g in range(N_OUT_DMAS):
                gt = opool.tile([TILE, TILES_PER_GROUP * N_PX], f32, tag=f"g{g}")
                for j2 in range(TILES_PER_GROUP // 2):
                    ps = pspool.tile([TILE, 2 * PXP], f32)
                    for h in range(2):
                        j = 2 * j2 + h
                        t = g * TILES_PER_GROUP + j
                        off = PXP + t * TILE
                        nc.tensor.matmul(
                            ps[:, h * PXP : h * PXP + PXP],
                            xt0[:, off : off + TILE],
                            wt,
                            start=True,
                            stop=True,
                        )
                    # one strided copy moves both 242-col blocks of the bank
                    src = ps[:].rearrange("p (h c) -> p h c", h=2)[:, :, :N_PX]
                    dst = gt[:, 2 * j2 * N_PX : (2 * j2 + 2) * N_PX].rearrange(
                        "p (h c) -> p h c", h=2
                    )
                    if (g * 4 + j2) % 2 == 0:
                        nc.scalar.copy(dst, src)
                    else:
                        nc.vector.tensor_copy(dst, src)
                nc.sync.dma_start(
                    out_g[g],
                    gt[:].rearrange("p (j c) -> p j c", j=TILES_PER_GROUP),
                )

    nc.compile()
    return nc


def _get_nc():
    if "nc" not in _BUILT:
        _BUILT["nc"] = _build_nc()
    return _BUILT["nc"]


def _pack_host(S_re, S_im, D_re, D_im, tau):
    """Build per-core input maps: transposed feature matrix + weight matrix."""
    Dr = np.asarray(D_re, dtype=np.float32)
    Di = np.asarray(D_im, dtype=np.float32)
    tau = np.asarray(tau, dtype=np.float32)

    Wr = (Dr[:, None, :] * Dr[None, :, :] + Di[:, None, :] * Di[None, :, :])
    Wi = (Di[:, None, :] * Dr[None, :, :] - Dr[:, None, :] * Di[None, :, :])
    W = np.empty((K_FEAT, PXP), dtype=np.float32)
    W[:, N_PX:] = 0.0
    W[:16, :N_PX] = Wr.reshape(16, N_PX)
    W[16:32, :N_PX] = Wi.reshape(16, N_PX)
    W[32, :N_PX] = -tau

    X = np.empty((K_FEAT, N_SAMPLES), dtype=np.float32)
    X[:16] = np.asarray(S_re, dtype=np.float32).reshape(N_SAMPLES, 16).T
    X[16:32] = np.asarray(S_im, dtype=np.float32).reshape(N_SAMPLES, 16).T
    X[32] = 1.0

    in_maps = []
    for i in range(N_CORES):
        xtw = np.empty((K_FEAT, PXP + N_LOC), dtype=np.float32)
        xtw[:, :PXP] = W
        xtw[:, PXP:] = X[:, i * N_LOC : (i + 1) * N_LOC]
        in_maps.append({"xTw": xtw})
    return in_maps


def _run(inputs, trace=False):
    from concourse.bass_utils import run_bass_kernel_spmd

    nc = _get_nc()
    in_maps = _pack_host(**inputs)
    res = run_bass_kernel_spmd(
        nc, in_maps, list(range(N_CORES)), trace=trace
    )
    out = np.concatenate([res.results[i]["out"] for i in range(N_CORES)], axis=0)
    return out, res


def kernel(**inputs) -> np.ndarray:
    out, _ = _run(inputs, trace=False)
    return out
